# revision 70
# baseline (speedup 1.0000x reference)
"""Trainium2 Bass kernel for nn_MultiHeadAttention_8667244003725.

B=4, S=1024, E=1024, H=16, D=64.  Reference:
  q/k/v = einsum('bse,hed->bhsd', x, W{q,k,v})
  scores = q@k^T/sqrt(D), causal mask, softmax
  heads -> concat (B,S,E);  out = W_O @ concat  (contracts over SEQUENCE dim)
  returns (B, E, E).

Sharding: 8 cores = 4 batches x 2 head-groups (8 heads each).  Because the
output projection contracts over the sequence dim, sharding heads shards the
output columns: core c computes out[b, :, 512*g : 512*g+512] with b=c//2,
g=c%2.  No collectives.

Design notes (evidence-driven, from perfetto traces of each revision;
the v1 baseline was 173us, this version ~126-128us):
 - v1 ran the whole attention+projection phase at the HAM-throttled 1.2 GHz
   PE clock (every MM exactly 2x the warm model) and burned 17.6us in PE
   transposes.  Fixes, in rough order of impact:
   * q-major AV dataflow (stationary = exp'd score block [128k,128q],
     moving = [V|ones] [128k,65]): head outputs land q-major with the
     softmax denominator in column 64 -> zero PE transposes, C written
     directly by reciprocal + tensor_scalar_mul.
   * V (not Q/K) is the startup phase, ec-major across 4 concurrent psum
     chains: every arriving (xb, wv) DMA chunk enables 4 matmuls, so the
     DMA-trickle window stays dense and HAM un-throttles early.  Junk
     matmuls at t=0 plus deterministic "heater" blocks between early ec
     groups bridge the arrival gaps (in-order PE queue: filler after a
     stalled MM is useless, so it must be placed *before* the stalls).
   * software pipeline: scores/exp of block kb overlap AV of kb-1; the
     next pair's Q/K (later, projection slices) zip into the attention
     stream as MM-channel filler.  AV matmuls are emitted as contiguous
     bursts: each MM costs max(prev_MM, own_LDW), so small-N MMs stay
     grouped (their ~30ns FWL weight loads pipeline) and only one
     small->big LDW transition is paid per step.
   * scores: stationary = KT block [64,128] at partition base 0/64 per
     head of the pair -> auto row-tile positions (0,0)/(64,0); adjacent
     emission makes the two heads' MMs run concurrently (~2x).
   * DMA: ~650ns per dma_start on a sequencer and one ring per trigger, so
     inputs go as [128,512] pieces spread across the sync/scalar/gpsimd
     queues (xb halves split sync/gpsimd); wv issues first (V first);
     the exp-table warm ACT is emitted after the weight triggers.
   * output stored bf16 (host casts back to f32; adds ~0.04% rel err) and
     the trailing projection half runs at N=256 where weight loads
     self-hide; final out-DMAs round-robin across three trigger queues.
 - psum: psQ 2 + psS 4 + psAV 2 = 8 banks; AV packs 4 interleaved
   accumulation chains (2 qb x 2 heads) per bank, start=True only on the
   first MM per bank generation (has_written clear is whole-bank).
"""

import sys

if '/opt/trn_rl_repo' not in sys.path:
    sys.path.insert(0, '/opt/trn_rl_repo')

import numpy as np

import concourse.bass as bass
import concourse.mybir as mybir
import concourse.tile as tile

F32 = mybir.dt.float32
AF = mybir.ActivationFunctionType

S = 1024          # sequence
E = 1024          # embed
D = 64            # head dim
HC = 8            # heads per core
NO = 512          # output columns per core
NJUNK = 8         # warm-up matmuls


def _split_sync_waits(nc, limit=1):
    """The walrus build in this env rejects >1 sem-wait per instruction.
    Hoist excess waits onto preceding same-engine no-ops (same queue, so
    program order preserves the wait semantics)."""
    n = 0
    for f in nc.m.functions:
        for bb in f.blocks:
            out = []
            for ins in bb.instructions:
                si = ins.sync_info
                waits = list(si.on_wait) if si is not None else []
                if len(waits) > limit:
                    excess, keep = waits[:-limit], waits[-limit:]
                    for i in range(0, len(excess), limit):
                        grp = excess[i:i + limit]
                        n += 1
                        out.append(mybir.InstNoOp(
                            name=f'I-synsplit-{n}', ins=[], outs=[],
                            engine=ins.engine,
                            sync_info=mybir.SyncInfo(on_wait=list(grp),
                                                     on_update=[])))
                    si.on_wait = keep
                out.append(ins)
            bb.instructions = out
    return n


def build_nc(split_waits=True):
    nc = bass.Bass()
    BF = mybir.dt.bfloat16
    xb = nc.dram_tensor('xb', [E, S], BF, kind='ExternalInput')   # x[b]^T
    wq = nc.dram_tensor('wq', [E, HC * D], BF, kind='ExternalInput')
    wk = nc.dram_tensor('wk', [E, HC * D], BF, kind='ExternalInput')
    wv = nc.dram_tensor('wv', [E, HC * D], BF, kind='ExternalInput')
    wo = nc.dram_tensor('wo', [E, E], BF, kind='ExternalInput')   # W_O^T
    out = nc.dram_tensor('out', [E, NO], BF, kind='ExternalOutput')

    with tile.TileContext(nc) as tc:
        _emit(nc, tc, xb, wq, wk, wv, wo, out)
    if split_waits:
        _split_sync_waits(nc)
    return nc


def _zip_emit(*streams):
    """streams: lists of (cost, closure).  Emit all items, interleaving the
    streams proportionally to cumulative cost, preserving per-stream order."""
    streams = [list(s) for s in streams if s]
    tot = [sum(c for c, _ in s) or 1.0 for s in streams]
    done = [0.0] * len(streams)
    idx = [0] * len(streams)
    while True:
        best, bf = -1, None
        for i, s in enumerate(streams):
            if idx[i] >= len(s):
                continue
            frac = done[i] / tot[i]
            if bf is None or frac < bf:
                best, bf = i, frac
        if best < 0:
            break
        c, fn = streams[best][idx[best]]
        idx[best] += 1
        done[best] += c
        fn()


def _emit(nc, tc, xb, wq, wk, wv, wo, out):
    BF = mybir.dt.bfloat16

    from contextlib import ExitStack
    es = ExitStack()
    constp = es.enter_context(tc.tile_pool(name='const', bufs=1))
    bigT = es.enter_context(tc.tile_pool(name='bigT', bufs=2))
    qkp = es.enter_context(tc.tile_pool(name='qk', bufs=1))
    vallp = es.enter_context(tc.tile_pool(name='vall', bufs=1))
    cp = es.enter_context(tc.tile_pool(name='cbuf', bufs=1))
    pexpp = es.enter_context(tc.tile_pool(name='pexp', bufs=8))
    rlp = es.enter_context(tc.tile_pool(name='rl', bufs=4))
    ostr = es.enter_context(tc.tile_pool(name='ostr', bufs=3))
    psQ = es.enter_context(tc.tile_pool(name='psQ', bufs=2, space='PSUM'))
    psS = es.enter_context(tc.tile_pool(name='psS', bufs=4, space='PSUM'))
    psAV = es.enter_context(tc.tile_pool(name='psAV', bufs=2, space='PSUM'))

    # ---- constants --------------------------------------------------------
    junk = constp.tile([128, 512], BF, tag='junk')
    nc.gpsimd.memset(junk[:], 0.001)

    # ---- PE warm-up: dense junk matmuls while input DMAs land ------------
    jt = psQ.tile([128, 512], F32, tag='q', name='junkps')
    for i in range(NJUNK):
        nc.tensor.matmul(jt[:], junk[:, 0:128], junk[:],
                         start=True, stop=True)

    # ---- input DMAs -------------------------------------------------------
    # Trigger issue costs ~650ns per dma_start on a sequencer, so spread:
    # xb chunks on the sync queue, wq/wk interleaved on the scalar queue
    # (Q/K chains consume them ec-major), wv + wo on the gpsimd SWDGE queue.
    # Per-chunk triggers (one trigger's data streams through one ring, so
    # keep pieces small and spread queues): xb halves on sync, wq/wk
    # interleaved then wv on scalar, wo on gpsimd.
    xTall = bigT.tile([128, 8 * S], BF, tag='bigT', name='xTall')
    for ec in range(8):
        for sc in range(2):
            eng = nc.sync if sc == 0 else nc.gpsimd
            eng.dma_start(
                xTall[:, ec * S + sc * 512:ec * S + (sc + 1) * 512],
                xb[ec * 128:(ec + 1) * 128, sc * 512:(sc + 1) * 512])
    xT = [xTall[:, ec * S:(ec + 1) * S] for ec in range(8)]

    wpool = tc.tile_pool(name='wts', bufs=1)
    wp = es.enter_context(wpool)
    wqall = wp.tile([128, 8 * HC * D], BF, tag='wqall', name='wqall')
    wkall = wp.tile([128, 8 * HC * D], BF, tag='wkall', name='wkall')
    wvall = wp.tile([128, 8 * HC * D], BF, tag='wvall', name='wvall')
    # wv first on scalar (V is the startup phase) then wq; wk rides the
    # sync queue after the xb halves — balances trigger issue (~650ns
    # each) across all three queues so every weight lands by ~12us.
    wqt, wkt, wvt = [], [], []
    for ec in range(8):
        sl = wvall[:, ec * HC * D:(ec + 1) * HC * D]
        nc.scalar.dma_start(sl, wv[ec * 128:(ec + 1) * 128, :])
        wvt.append(sl)
    for ec in range(8):
        sl = wqall[:, ec * HC * D:(ec + 1) * HC * D]
        nc.scalar.dma_start(sl, wq[ec * 128:(ec + 1) * 128, :])
        wqt.append(sl)
        sk = wkall[:, ec * HC * D:(ec + 1) * HC * D]
        nc.sync.dma_start(sk, wk[ec * 128:(ec + 1) * 128, :])
        wkt.append(sk)
    # warm the ACT exp table (after the weight triggers — table load is
    # 1.3us and must not delay them; first real exp is ~20us in)
    warm = constp.tile([1, 2], F32, tag='warm')
    nc.scalar.activation(warm[:], junk[0:1, 0:2], AF.Exp, scale=0.125)
    # gpsimd queue: constants then the late-needed W_O^T
    tri = constp.tile([128, 128], BF, tag='tri')
    nc.gpsimd.memset(tri[:], 1.0)
    nc.gpsimd.affine_select(
        out=tri[:], in_=tri[:], compare_op=mybir.AluOpType.is_ge,
        fill=0.0, base=0, channel_multiplier=-1, pattern=[[1, 128]])
    ones8 = constp.tile([128, 8], BF, tag='ones8')
    nc.gpsimd.memset(ones8[:], 1.0)
    WOTall = bigT.tile([128, 8 * E], BF, tag='bigT', name='WOTall')
    for sc in range(8):
        nc.gpsimd.dma_start(WOTall[:, sc * E:(sc + 1) * E],
                            wo[sc * 128:(sc + 1) * 128, :])
    WOT = [WOTall[:, sc * E:(sc + 1) * E] for sc in range(8)]

    # ---- persistent SBUF tensors -----------------------------------------
    QT2 = [qkp.tile([128, S], BF, tag=f'q{p}', name=f'QT2_{p}')
           for p in range(4)]
    KT2 = [qkp.tile([128, S], BF, tag=f'k{p}', name=f'KT2_{p}')
           for p in range(4)]
    Vall = [vallp.tile([128, HC * (D + 1)], BF, tag=f'v{st}',
                       name=f'Vall{st}') for st in range(8)]
    C = [cp.tile([128, NO], BF, tag=f'c{st}', name=f'C{st}')
         for st in range(8)]

    # ---- stream generators ------------------------------------------------
    def qk_items(pairs, pool_tags, scs=(0, 1)):
        """Q+K for the given head pairs: per sc-half, one chain per
        (pair, q|k) interleaved ec-major.  pool_tags: list of (pool, tag)
        per chain, len = 2*len(pairs)."""
        items = []
        for sc in scs:
            pss = {}

            def alloc(sc=sc, pss=pss):
                for i, p in enumerate(pairs):
                    pool, tag = pool_tags[2 * i]
                    pss[(p, 'q')] = pool.tile([128, 512], F32, tag=tag,
                                              name=f'qk_q{p}_{sc}')
                    pool, tag = pool_tags[2 * i + 1]
                    pss[(p, 'k')] = pool.tile([128, 512], F32, tag=tag,
                                              name=f'qk_k{p}_{sc}')
            items.append((0.0, alloc))
            for ec in range(8):
                def mm(ec=ec, sc=sc, pss=pss):
                    for p in pairs:
                        nc.tensor.matmul(
                            pss[(p, 'q')][:],
                            wqt[ec][:, p * 128:(p + 1) * 128],
                            xT[ec][:, sc * 512:(sc + 1) * 512],
                            start=(ec == 0), stop=(ec == 7))
                        nc.tensor.matmul(
                            pss[(p, 'k')][:],
                            wkt[ec][:, p * 128:(p + 1) * 128],
                            xT[ec][:, sc * 512:(sc + 1) * 512],
                            start=(ec == 0), stop=(ec == 7))
                items.append((len(pairs) * 430.0, mm))

            def cp_(sc=sc, pss=pss):
                for p in pairs:
                    nc.vector.tensor_copy(QT2[p][:, sc * 512:(sc + 1) * 512],
                                          pss[(p, 'q')][:])
                    nc.vector.tensor_copy(KT2[p][:, sc * 512:(sc + 1) * 512],
                                          pss[(p, 'k')][:])
            items.append((0.0, cp_))
        return items

    def v_wave_items(sts, pool_tags):
        """V for 4 st-chunks, ec-major across 4 concurrent psum chains —
        every arriving (xb, wv) ec chunk enables 4 matmuls, so the DMA
        trickle phase stays dense."""
        items = []
        st_ = {}

        def alloc():
            st_['ps'] = {st: pool.tile([128, 512], F32, tag=tag,
                                       name=f'v_{st}')
                         for st, (pool, tag) in zip(sts, pool_tags)}
        items.append((0.0, alloc))
        for ec in range(8):
            def mm(ec=ec):
                for st in sts:
                    nc.tensor.matmul(st_['ps'][st][:],
                                     xT[ec][:, st * 128:(st + 1) * 128],
                                     wvt[ec],
                                     start=(ec == 0), stop=(ec == 7))
            items.append((4 * 216.0, mm))

        def fin():
            for st in sts:
                v3 = Vall[st][:].rearrange('p (h d) -> p h d', h=HC)
                nc.vector.tensor_copy(
                    v3[:, :, 0:D],
                    st_['ps'][st][:].rearrange('p (h d) -> p h d', h=HC))
                nc.vector.tensor_copy(
                    v3[:, :, D:D + 1],
                    ones8[:].rearrange('p (h o) -> p h o', o=1))
        items.append((0.0, fin))
        return items

    def attn_items(p, qcs=(0, 1)):
        """Attention for head pair p (heads 2p, 2p+1).  kb-outer, one-step
        software pipeline: AV matmuls of block kb are emitted after the
        scores of kb+1 (so the exp has a full step + zip filler of slack),
        and as ONE MATMUL PER ITEM so their LDWEIGHTS hide under the big
        interleaved QKV/proj matmuls."""
        items = []
        for qc in qcs:
            nkb = 4 * qc + 4
            st_ = {}

            def qc_alloc(qc=qc, st_=st_):
                # two packed AV banks: qb 0,1 and qb 2,3 (cols 130 each)
                st_['av'] = [psAV.tile([128, 512], F32, tag='av',
                                       name=f'av{p}_{qc}_{i}')
                             for i in range(2)]
                st_['first'] = [True, True]
                st_['pexp'] = {}
            items.append((0.0, qc_alloc))

            def score(kb, qc=qc, st_=st_):
                off = max(0, 128 * kb - 512 * qc)
                for hl in range(2):
                    stp = psS.tile([128, 512], F32, tag='s',
                                   name=f'st{p}_{qc}_{kb}_{hl}')
                    nc.tensor.matmul(
                        stp[:, off:512],
                        KT2[p][64 * hl:64 * hl + 64,
                               kb * 128:(kb + 1) * 128],
                        QT2[p][64 * hl:64 * hl + 64,
                               qc * 512 + off:(qc + 1) * 512],
                        start=True, stop=True)
                    pexp = pexpp.tile([128, 512], BF, tag='pe',
                                      name=f'pe{p}_{qc}_{kb}_{hl}')
                    nc.scalar.activation(pexp[:, off:512],
                                         stp[:, off:512],
                                         AF.Exp, scale=0.125)
                    if kb >= 4 * qc:  # diagonal block: causal mask
                        nc.vector.tensor_mul(pexp[:, off:off + 128],
                                             pexp[:, off:off + 128],
                                             tri[:])
                    st_['pexp'][(kb, hl)] = pexp

            def av_mm(kb, qb, hl, qc=qc, st_=st_):
                gq = 4 * qc + qb
                bank = st_['av'][qb // 2]
                cb = (qb % 2) * 130
                last = (kb == 4 * qc + qb)
                nc.tensor.matmul(
                    bank[:, cb + hl * 65:cb + hl * 65 + 65],
                    st_['pexp'][(kb, hl)][:, qb * 128:(qb + 1) * 128],
                    Vall[kb][:, (2 * p + hl) * (D + 1):
                             (2 * p + hl + 1) * (D + 1)],
                    start=st_['first'][qb // 2], stop=last,
                    skip_group_check=True)
                st_['first'][qb // 2] = False
                if last:
                    rl = rlp.tile([128, 1], F32, tag='rl')
                    nc.vector.reciprocal(
                        rl[:], bank[:, cb + hl * 65 + D:
                                    cb + hl * 65 + D + 1])
                    nc.vector.tensor_scalar_mul(
                        C[gq][:, (2 * p + hl) * D:(2 * p + hl + 1) * D],
                        bank[:, cb + hl * 65:cb + hl * 65 + D],
                        rl[:])
                    if hl == 1:
                        del st_['pexp'][(kb, 0)], st_['pexp'][(kb, 1)]

            def av_step(kb, qc=qc, av_mm=av_mm):
                # non-diagonal qb first (diag stationary needs the mask);
                # one burst item — small MMs stay contiguous so only one
                # small->big LDW transition is paid per step
                qb_start = max(0, kb - 4 * qc)
                for qb in list(range(qb_start + 1, 4)) + [qb_start]:
                    for hl in range(2):
                        av_mm(kb, qb, hl)

            for kb in range(nkb):
                items.append((430.0, lambda kb=kb, score=score: score(kb)))
                if kb > 0:
                    items.append(((4 - max(0, kb - 1 - 4 * qc)) * 2 * 65.0,
                                  lambda kb=kb, av_step=av_step:
                                  av_step(kb - 1)))
            items.append(((4 - max(0, nkb - 1 - 4 * qc)) * 2 * 65.0,
                          lambda nkb=nkb, av_step=av_step:
                          av_step(nkb - 1)))
        return items

    def proj_items(c0, c1, pools=None):
        """Output projection for out columns [c0:c1); it-chains rotate
        across `pools` [(pool, tag), ...] for pipeline depth."""
        items = []
        W = c1 - c0
        mmcost = max(97.0, W / 128 * 56.0)
        if pools is None:
            pools = [(psQ, 'q')]
        for it in range(8):
            st_ = {}

            def alloc(it=it, st_=st_):
                pool, tag = pools[it % len(pools)]
                st_['ps'] = pool.tile([128, 512], F32, tag=tag,
                                      name=f'pj{it}_{c0}')
            items.append((0.0, alloc))
            for s4 in range(2):
                def mm4(it=it, s4=s4, st_=st_):
                    for sc in range(4 * s4, 4 * s4 + 4):
                        nc.tensor.matmul(st_['ps'][:, 0:W],
                                         WOT[sc][:, it * 128:(it + 1) * 128],
                                         C[sc][:, c0:c1],
                                         start=(sc == 0), stop=(sc == 7))
                items.append((4 * mmcost, mm4))

            def fin(it=it, st_=st_):
                ys = ostr.tile([128, W], BF, tag='ys')
                nc.vector.tensor_copy(ys[:], st_['ps'][:, 0:W])
                eng = (nc.sync, nc.scalar, nc.gpsimd)[it % 3]
                eng.dma_start(out[it * 128:(it + 1) * 128, c0:c1], ys[:])
            items.append((200.0, fin))
        return items



    def heater_items(n):
        """HAM-warming junk matmuls on an own psAV-tag bank; zipped into
        DMA-gated phases so the PE activity window never goes idle."""
        st_ = {}

        def alloc():
            st_['t'] = psAV.tile([128, 512], F32, tag='av', name='heat')
        items = [(0.0, alloc)]
        for i in range(n):
            def mm():
                nc.tensor.matmul(st_['t'][:], junk[:, 0:128], junk[:],
                                 start=True, stop=True)
            items.append((210.0, mm))
        return items

    # ---- schedule ---------------------------------------------------------
    # V first: ec-major waves keep the PE dense through the DMA trickle.
    # Deterministic heater blocks between the early ec groups fill the
    # chunk-arrival gaps (front-loaded: arrivals accelerate as rings ramp).
    qk = lambda p: qk_items((p,), [(psQ, 'q'), (psQ, 'q')])
    # Merged startup block: wave1 V (st 0-3) + the sc0 halves of Q0/K0,
    # all ec-major — 6 matmuls per arriving (xb, w) ec chunk — plus
    # front-loaded heaters.  qk0-sc0 chains borrow the other 2 psS banks.
    w1 = v_wave_items((0, 1, 2, 3),
                      [(psQ, 'q'), (psQ, 'q'), (psS, 's'), (psS, 's')])
    q0a = qk_items((0,), [(psS, 's'), (psS, 's')], scs=(0,))
    he = heater_items(14)
    he[0][1]()                                   # heater psum alloc
    hmms = [fn for _, fn in he[1:]]
    w1[0][1]()                                   # wave1 psum alloc
    q0a[0][1]()                                  # qk0-sc0 psum alloc
    fills = [4, 3, 3, 2, 1, 1, 0, 0]
    hi = 0
    for i in range(8):
        w1[1 + i][1]()                           # 4 V MMs for this ec
        q0a[1 + i][1]()                          # Q0/K0 sc0 MMs
        for _ in range(fills[i]):
            hmms[hi](); hi += 1
    w1[9][1]()                                   # wave1 copies
    q0a[9][1]()                                  # qk0-sc0 copies
    # wave2 ‖ p0 qc0 (its scores need only the sc0 Q/K halves; AV needs
    # Vall[0..3] from wave1).  wave2 is dense (all data arrived) so it
    # runs 2 chains at a time on psQ, leaving psS to the attention scores.
    a0 = attn_items(0)
    _zip_emit(v_wave_items((4, 5), [(psQ, 'q'), (psQ, 'q')])
              + v_wave_items((6, 7), [(psQ, 'q'), (psQ, 'q')]),
              a0[:9])
    _zip_emit(a0[9:], qk_items((0,), [(psQ, 'q'), (psQ, 'q')], scs=(1,))
              + qk(1))
    _zip_emit(attn_items(1), qk(2))                  # p1 ‖ Q2K2
    _zip_emit(attn_items(2), qk(3), proj_items(0, 128))
    _zip_emit(attn_items(3), proj_items(128, 256))   # p3 ‖ proj(pair 1)
    # trailing half at N=256 (MM 110ns > LDW 97ns: weight loads self-hide);
    # attention psum pools are free now — rotate chains across psQ/psS so
    # copies hide behind the next chains
    for _, fn in proj_items(256, 512, pools=[(psQ, 'q'), (psS, 's')]):
        fn()
    es.close()

    es.close()


_NC_CACHE = None


def _get_nc():
    global _NC_CACHE
    if _NC_CACHE is None:
        _NC_CACHE = build_nc()
    return _NC_CACHE


def make_in_maps(x, Wq, Wk, Wv, W_O):
    import ml_dtypes
    bf = ml_dtypes.bfloat16
    x = np.asarray(x, np.float32)
    xT_by_b = [np.ascontiguousarray(x[b].T.astype(bf)) for b in range(4)]
    W_O = np.ascontiguousarray(np.asarray(W_O, np.float32).T.astype(bf))
    in_maps = []
    for c in range(8):
        b, g = c // 2, c % 2
        hsl = slice(HC * g, HC * g + HC)
        in_maps.append({
            'xb': xT_by_b[b],
            'wq': np.ascontiguousarray(
                np.asarray(Wq, np.float32)[hsl].transpose(1, 0, 2)
                .reshape(E, HC * D).astype(bf)),
            'wk': np.ascontiguousarray(
                np.asarray(Wk, np.float32)[hsl].transpose(1, 0, 2)
                .reshape(E, HC * D).astype(bf)),
            'wv': np.ascontiguousarray(
                np.asarray(Wv, np.float32)[hsl].transpose(1, 0, 2)
                .reshape(E, HC * D).astype(bf)),
            'wo': W_O,
        })
    return in_maps


def kernel(x, Wq, Wk, Wv, W_O):
    from concourse.bass_utils import run_bass_kernel_spmd
    nc = _get_nc()
    in_maps = make_in_maps(x, Wq, Wk, Wv, W_O)
    res = run_bass_kernel_spmd(nc, in_maps, list(range(8)))
    full = np.empty((4, E, E), np.float32)
    for c in range(8):
        b, g = c // 2, c % 2
        full[b, :, NO * g:NO * g + NO] = res.results[c]['out']
    return full


# revision 72
# speedup vs baseline: 1.0166x; 1.0166x over previous
"""Trainium2 Bass kernel for nn_MultiHeadAttention_8667244003725.

B=4, S=1024, E=1024, H=16, D=64.  Reference:
  q/k/v = einsum('bse,hed->bhsd', x, W{q,k,v})
  scores = q@k^T/sqrt(D), causal mask, softmax
  heads -> concat (B,S,E);  out = W_O @ concat  (contracts over SEQUENCE dim)
  returns (B, E, E).

Sharding: 8 cores = 4 batches x 2 head-groups (8 heads each).  Because the
output projection contracts over the sequence dim, sharding heads shards the
output columns: core c computes out[b, :, 512*g : 512*g+512] with b=c//2,
g=c%2.  No collectives.

Design notes (evidence-driven, from perfetto traces of each revision;
the v1 baseline was 173us, this version ~126-128us):
 - v1 ran the whole attention+projection phase at the HAM-throttled 1.2 GHz
   PE clock (every MM exactly 2x the warm model) and burned 17.6us in PE
   transposes.  Fixes, in rough order of impact:
   * q-major AV dataflow (stationary = exp'd score block [128k,128q],
     moving = [V|ones] [128k,65]): head outputs land q-major with the
     softmax denominator in column 64 -> zero PE transposes, C written
     directly by reciprocal + tensor_scalar_mul.
   * V (not Q/K) is the startup phase, ec-major across 4 concurrent psum
     chains: every arriving (xb, wv) DMA chunk enables 4 matmuls, so the
     DMA-trickle window stays dense and HAM un-throttles early.  Junk
     matmuls at t=0 plus deterministic "heater" blocks between early ec
     groups bridge the arrival gaps (in-order PE queue: filler after a
     stalled MM is useless, so it must be placed *before* the stalls).
   * software pipeline: scores/exp of block kb overlap AV of kb-1; the
     next pair's Q/K (later, projection slices) zip into the attention
     stream as MM-channel filler.  AV matmuls are emitted as contiguous
     bursts: each MM costs max(prev_MM, own_LDW), so small-N MMs stay
     grouped (their ~30ns FWL weight loads pipeline) and only one
     small->big LDW transition is paid per step.
   * scores: stationary = KT block [64,128] at partition base 0/64 per
     head of the pair -> auto row-tile positions (0,0)/(64,0); adjacent
     emission makes the two heads' MMs run concurrently (~2x).
   * DMA: ~650ns per dma_start on a sequencer and one ring per trigger, so
     inputs go as [128,512] pieces spread across the sync/scalar/gpsimd
     queues (xb halves split sync/gpsimd); wv issues first (V first);
     the exp-table warm ACT is emitted after the weight triggers.
   * output stored bf16 (host casts back to f32; adds ~0.04% rel err) and
     the trailing projection half runs at N=256 where weight loads
     self-hide; final out-DMAs round-robin across three trigger queues.
 - psum: psQ 2 + psS 4 + psAV 2 = 8 banks; AV packs 4 interleaved
   accumulation chains (2 qb x 2 heads) per bank, start=True only on the
   first MM per bank generation (has_written clear is whole-bank).
"""

import sys

if '/opt/trn_rl_repo' not in sys.path:
    sys.path.insert(0, '/opt/trn_rl_repo')

import numpy as np

import concourse.bass as bass
import concourse.mybir as mybir
import concourse.tile as tile

F32 = mybir.dt.float32
AF = mybir.ActivationFunctionType

S = 1024          # sequence
E = 1024          # embed
D = 64            # head dim
HC = 8            # heads per core
NO = 512          # output columns per core
NJUNK = 8         # warm-up matmuls


def _split_sync_waits(nc, limit=1):
    """The walrus build in this env rejects >1 sem-wait per instruction.
    Hoist excess waits onto preceding same-engine no-ops (same queue, so
    program order preserves the wait semantics)."""
    n = 0
    for f in nc.m.functions:
        for bb in f.blocks:
            out = []
            for ins in bb.instructions:
                si = ins.sync_info
                waits = list(si.on_wait) if si is not None else []
                if len(waits) > limit:
                    excess, keep = waits[:-limit], waits[-limit:]
                    for i in range(0, len(excess), limit):
                        grp = excess[i:i + limit]
                        n += 1
                        out.append(mybir.InstNoOp(
                            name=f'I-synsplit-{n}', ins=[], outs=[],
                            engine=ins.engine,
                            sync_info=mybir.SyncInfo(on_wait=list(grp),
                                                     on_update=[])))
                    si.on_wait = keep
                out.append(ins)
            bb.instructions = out
    return n


def build_nc(split_waits=True):
    nc = bass.Bass()
    BF = mybir.dt.bfloat16
    xb = nc.dram_tensor('xb', [E, S], BF, kind='ExternalInput')   # x[b]^T
    wq = nc.dram_tensor('wq', [E, HC * D], BF, kind='ExternalInput')
    wk = nc.dram_tensor('wk', [E, HC * D], BF, kind='ExternalInput')
    wv = nc.dram_tensor('wv', [E, HC * D], BF, kind='ExternalInput')
    wo = nc.dram_tensor('wo', [E, E], BF, kind='ExternalInput')   # W_O^T
    out = nc.dram_tensor('out', [E, NO], BF, kind='ExternalOutput')

    with tile.TileContext(nc) as tc:
        _emit(nc, tc, xb, wq, wk, wv, wo, out)
    if split_waits:
        _split_sync_waits(nc)
    return nc


def _zip_emit(*streams):
    """streams: lists of (cost, closure).  Emit all items, interleaving the
    streams proportionally to cumulative cost, preserving per-stream order."""
    streams = [list(s) for s in streams if s]
    tot = [sum(c for c, _ in s) or 1.0 for s in streams]
    done = [0.0] * len(streams)
    idx = [0] * len(streams)
    while True:
        best, bf = -1, None
        for i, s in enumerate(streams):
            if idx[i] >= len(s):
                continue
            frac = done[i] / tot[i]
            if bf is None or frac < bf:
                best, bf = i, frac
        if best < 0:
            break
        c, fn = streams[best][idx[best]]
        idx[best] += 1
        done[best] += c
        fn()


def _emit(nc, tc, xb, wq, wk, wv, wo, out):
    BF = mybir.dt.bfloat16

    from contextlib import ExitStack
    es = ExitStack()
    constp = es.enter_context(tc.tile_pool(name='const', bufs=1))
    bigT = es.enter_context(tc.tile_pool(name='bigT', bufs=2))
    qkp = es.enter_context(tc.tile_pool(name='qk', bufs=1))
    vallp = es.enter_context(tc.tile_pool(name='vall', bufs=1))
    cp = es.enter_context(tc.tile_pool(name='cbuf', bufs=1))
    pexpp = es.enter_context(tc.tile_pool(name='pexp', bufs=8))
    rlp = es.enter_context(tc.tile_pool(name='rl', bufs=4))
    ostr = es.enter_context(tc.tile_pool(name='ostr', bufs=3))
    psQ = es.enter_context(tc.tile_pool(name='psQ', bufs=2, space='PSUM'))
    psS = es.enter_context(tc.tile_pool(name='psS', bufs=4, space='PSUM'))
    psAV = es.enter_context(tc.tile_pool(name='psAV', bufs=2, space='PSUM'))

    # ---- constants --------------------------------------------------------
    junk = constp.tile([128, 512], BF, tag='junk')
    nc.gpsimd.memset(junk[:], 0.001)

    # ---- PE warm-up: dense junk matmuls while input DMAs land ------------
    jt = psQ.tile([128, 512], F32, tag='q', name='junkps')
    for i in range(NJUNK):
        nc.tensor.matmul(jt[:], junk[:, 0:128], junk[:],
                         start=True, stop=True)

    # ---- input DMAs -------------------------------------------------------
    # Trigger issue costs ~650ns per dma_start on a sequencer, so spread:
    # xb chunks on the sync queue, wq/wk interleaved on the scalar queue
    # (Q/K chains consume them ec-major), wv + wo on the gpsimd SWDGE queue.
    # Per-chunk triggers (one trigger's data streams through one ring, so
    # keep pieces small and spread queues): xb halves on sync, wq/wk
    # interleaved then wv on scalar, wo on gpsimd.
    xTall = bigT.tile([128, 8 * S], BF, tag='bigT', name='xTall')
    xT = [xTall[:, ec * S:(ec + 1) * S] for ec in range(8)]

    wpool = tc.tile_pool(name='wts', bufs=1)
    wp = es.enter_context(wpool)
    wqall = wp.tile([128, 8 * HC * D], BF, tag='wqall', name='wqall')
    wkall = wp.tile([128, 8 * HC * D], BF, tag='wkall', name='wkall')
    wvall = wp.tile([128, 8 * HC * D], BF, tag='wvall', name='wvall')
    # Per-ec interleaved trigger order so each ec's full working set
    # (xb halves, wv, wq, wk) lands together: the merged startup block
    # consumes 6 matmuls per ec.  sync: xb-sc0, wk; scalar: wv, wq;
    # gpsimd: xb-sc1 (then consts + wo).
    wqt, wkt, wvt = [], [], []
    for ec in range(8):
        nc.sync.dma_start(
            xTall[:, ec * S:ec * S + 512],
            xb[ec * 128:(ec + 1) * 128, 0:512])
        sk = wkall[:, ec * HC * D:(ec + 1) * HC * D]
        nc.sync.dma_start(sk, wk[ec * 128:(ec + 1) * 128, :])
        wkt.append(sk)
        sl = wvall[:, ec * HC * D:(ec + 1) * HC * D]
        nc.scalar.dma_start(sl, wv[ec * 128:(ec + 1) * 128, :])
        wvt.append(sl)
        sq = wqall[:, ec * HC * D:(ec + 1) * HC * D]
        nc.scalar.dma_start(sq, wq[ec * 128:(ec + 1) * 128, :])
        wqt.append(sq)
        nc.gpsimd.dma_start(
            xTall[:, ec * S + 512:(ec + 1) * S],
            xb[ec * 128:(ec + 1) * 128, 512:1024])
    # warm the ACT exp table (after the weight triggers — table load is
    # 1.3us and must not delay them; first real exp is ~20us in)
    warm = constp.tile([1, 2], F32, tag='warm')
    nc.scalar.activation(warm[:], junk[0:1, 0:2], AF.Exp, scale=0.125)
    # gpsimd queue: constants then the late-needed W_O^T
    tri = constp.tile([128, 128], BF, tag='tri')
    nc.gpsimd.memset(tri[:], 1.0)
    nc.gpsimd.affine_select(
        out=tri[:], in_=tri[:], compare_op=mybir.AluOpType.is_ge,
        fill=0.0, base=0, channel_multiplier=-1, pattern=[[1, 128]])
    ones8 = constp.tile([128, 8], BF, tag='ones8')
    nc.gpsimd.memset(ones8[:], 1.0)
    WOTall = bigT.tile([128, 8 * E], BF, tag='bigT', name='WOTall')
    for sc in range(8):
        nc.gpsimd.dma_start(WOTall[:, sc * E:(sc + 1) * E],
                            wo[sc * 128:(sc + 1) * 128, :])
    WOT = [WOTall[:, sc * E:(sc + 1) * E] for sc in range(8)]

    # ---- persistent SBUF tensors -----------------------------------------
    QT2 = [qkp.tile([128, S], BF, tag=f'q{p}', name=f'QT2_{p}')
           for p in range(4)]
    KT2 = [qkp.tile([128, S], BF, tag=f'k{p}', name=f'KT2_{p}')
           for p in range(4)]
    Vall = [vallp.tile([128, HC * (D + 1)], BF, tag=f'v{st}',
                       name=f'Vall{st}') for st in range(8)]
    C = [cp.tile([128, NO], BF, tag=f'c{st}', name=f'C{st}')
         for st in range(8)]

    # ---- stream generators ------------------------------------------------
    def qk_items(pairs, pool_tags, scs=(0, 1)):
        """Q+K for the given head pairs: per sc-half, one chain per
        (pair, q|k) interleaved ec-major.  pool_tags: list of (pool, tag)
        per chain, len = 2*len(pairs)."""
        items = []
        for sc in scs:
            pss = {}

            def alloc(sc=sc, pss=pss):
                for i, p in enumerate(pairs):
                    pool, tag = pool_tags[2 * i]
                    pss[(p, 'q')] = pool.tile([128, 512], F32, tag=tag,
                                              name=f'qk_q{p}_{sc}')
                    pool, tag = pool_tags[2 * i + 1]
                    pss[(p, 'k')] = pool.tile([128, 512], F32, tag=tag,
                                              name=f'qk_k{p}_{sc}')
            items.append((0.0, alloc))
            for ec in range(8):
                def mm(ec=ec, sc=sc, pss=pss):
                    for p in pairs:
                        nc.tensor.matmul(
                            pss[(p, 'q')][:],
                            wqt[ec][:, p * 128:(p + 1) * 128],
                            xT[ec][:, sc * 512:(sc + 1) * 512],
                            start=(ec == 0), stop=(ec == 7))
                        nc.tensor.matmul(
                            pss[(p, 'k')][:],
                            wkt[ec][:, p * 128:(p + 1) * 128],
                            xT[ec][:, sc * 512:(sc + 1) * 512],
                            start=(ec == 0), stop=(ec == 7))
                items.append((len(pairs) * 430.0, mm))

            def cp_(sc=sc, pss=pss):
                for p in pairs:
                    nc.vector.tensor_copy(QT2[p][:, sc * 512:(sc + 1) * 512],
                                          pss[(p, 'q')][:])
                    nc.vector.tensor_copy(KT2[p][:, sc * 512:(sc + 1) * 512],
                                          pss[(p, 'k')][:])
            items.append((0.0, cp_))
        return items

    def v_wave_items(sts, pool_tags):
        """V for 4 st-chunks, ec-major across 4 concurrent psum chains —
        every arriving (xb, wv) ec chunk enables 4 matmuls, so the DMA
        trickle phase stays dense."""
        items = []
        st_ = {}

        def alloc():
            st_['ps'] = {st: pool.tile([128, 512], F32, tag=tag,
                                       name=f'v_{st}')
                         for st, (pool, tag) in zip(sts, pool_tags)}
        items.append((0.0, alloc))
        for ec in range(8):
            def mm(ec=ec):
                for st in sts:
                    nc.tensor.matmul(st_['ps'][st][:],
                                     xT[ec][:, st * 128:(st + 1) * 128],
                                     wvt[ec],
                                     start=(ec == 0), stop=(ec == 7))
            items.append((4 * 216.0, mm))

        def fin():
            for st in sts:
                v3 = Vall[st][:].rearrange('p (h d) -> p h d', h=HC)
                nc.vector.tensor_copy(
                    v3[:, :, 0:D],
                    st_['ps'][st][:].rearrange('p (h d) -> p h d', h=HC))
                nc.vector.tensor_copy(
                    v3[:, :, D:D + 1],
                    ones8[:].rearrange('p (h o) -> p h o', o=1))
        items.append((0.0, fin))
        return items

    def attn_items(p, qcs=(0, 1)):
        """Attention for head pair p (heads 2p, 2p+1).  kb-outer, one-step
        software pipeline: AV matmuls of block kb are emitted after the
        scores of kb+1 (so the exp has a full step + zip filler of slack),
        and as ONE MATMUL PER ITEM so their LDWEIGHTS hide under the big
        interleaved QKV/proj matmuls."""
        items = []
        for qc in qcs:
            nkb = 4 * qc + 4
            st_ = {}

            def qc_alloc(qc=qc, st_=st_):
                # two packed AV banks: qb 0,1 and qb 2,3 (cols 130 each)
                st_['av'] = [psAV.tile([128, 512], F32, tag='av',
                                       name=f'av{p}_{qc}_{i}')
                             for i in range(2)]
                st_['first'] = [True, True]
                st_['pexp'] = {}
            items.append((0.0, qc_alloc))

            def score(kb, qc=qc, st_=st_):
                off = max(0, 128 * kb - 512 * qc)
                for hl in range(2):
                    stp = psS.tile([128, 512], F32, tag='s',
                                   name=f'st{p}_{qc}_{kb}_{hl}')
                    nc.tensor.matmul(
                        stp[:, off:512],
                        KT2[p][64 * hl:64 * hl + 64,
                               kb * 128:(kb + 1) * 128],
                        QT2[p][64 * hl:64 * hl + 64,
                               qc * 512 + off:(qc + 1) * 512],
                        start=True, stop=True)
                    pexp = pexpp.tile([128, 512], BF, tag='pe',
                                      name=f'pe{p}_{qc}_{kb}_{hl}')
                    nc.scalar.activation(pexp[:, off:512],
                                         stp[:, off:512],
                                         AF.Exp, scale=0.125)
                    if kb >= 4 * qc:  # diagonal block: causal mask
                        nc.vector.tensor_mul(pexp[:, off:off + 128],
                                             pexp[:, off:off + 128],
                                             tri[:])
                    st_['pexp'][(kb, hl)] = pexp

            def av_mm(kb, qb, hl, qc=qc, st_=st_):
                gq = 4 * qc + qb
                bank = st_['av'][qb // 2]
                cb = (qb % 2) * 130
                last = (kb == 4 * qc + qb)
                nc.tensor.matmul(
                    bank[:, cb + hl * 65:cb + hl * 65 + 65],
                    st_['pexp'][(kb, hl)][:, qb * 128:(qb + 1) * 128],
                    Vall[kb][:, (2 * p + hl) * (D + 1):
                             (2 * p + hl + 1) * (D + 1)],
                    start=st_['first'][qb // 2], stop=last,
                    skip_group_check=True)
                st_['first'][qb // 2] = False
                if last:
                    rl = rlp.tile([128, 1], F32, tag='rl')
                    nc.vector.reciprocal(
                        rl[:], bank[:, cb + hl * 65 + D:
                                    cb + hl * 65 + D + 1])
                    nc.vector.tensor_scalar_mul(
                        C[gq][:, (2 * p + hl) * D:(2 * p + hl + 1) * D],
                        bank[:, cb + hl * 65:cb + hl * 65 + D],
                        rl[:])
                    if hl == 1:
                        del st_['pexp'][(kb, 0)], st_['pexp'][(kb, 1)]

            def av_step(kb, qc=qc, av_mm=av_mm):
                # non-diagonal qb first (diag stationary needs the mask);
                # one burst item — small MMs stay contiguous so only one
                # small->big LDW transition is paid per step
                qb_start = max(0, kb - 4 * qc)
                for qb in list(range(qb_start + 1, 4)) + [qb_start]:
                    for hl in range(2):
                        av_mm(kb, qb, hl)

            for kb in range(nkb):
                items.append((430.0, lambda kb=kb, score=score: score(kb)))
                if kb > 0:
                    items.append(((4 - max(0, kb - 1 - 4 * qc)) * 2 * 65.0,
                                  lambda kb=kb, av_step=av_step:
                                  av_step(kb - 1)))
            items.append(((4 - max(0, nkb - 1 - 4 * qc)) * 2 * 65.0,
                          lambda nkb=nkb, av_step=av_step:
                          av_step(nkb - 1)))
        return items

    def proj_items(c0, c1, pools=None):
        """Output projection for out columns [c0:c1); it-chains rotate
        across `pools` [(pool, tag), ...] for pipeline depth."""
        items = []
        W = c1 - c0
        mmcost = max(97.0, W / 128 * 56.0)
        if pools is None:
            pools = [(psQ, 'q')]
        for it in range(8):
            st_ = {}

            def alloc(it=it, st_=st_):
                pool, tag = pools[it % len(pools)]
                st_['ps'] = pool.tile([128, 512], F32, tag=tag,
                                      name=f'pj{it}_{c0}')
            items.append((0.0, alloc))
            for s4 in range(2):
                def mm4(it=it, s4=s4, st_=st_):
                    for sc in range(4 * s4, 4 * s4 + 4):
                        nc.tensor.matmul(st_['ps'][:, 0:W],
                                         WOT[sc][:, it * 128:(it + 1) * 128],
                                         C[sc][:, c0:c1],
                                         start=(sc == 0), stop=(sc == 7))
                items.append((4 * mmcost, mm4))

            def fin(it=it, st_=st_):
                ys = ostr.tile([128, W], BF, tag='ys')
                nc.vector.tensor_copy(ys[:], st_['ps'][:, 0:W])
                eng = (nc.sync, nc.scalar, nc.gpsimd)[it % 3]
                eng.dma_start(out[it * 128:(it + 1) * 128, c0:c1], ys[:])
            items.append((200.0, fin))
        return items



    def heater_items(n):
        """HAM-warming junk matmuls on an own psAV-tag bank; zipped into
        DMA-gated phases so the PE activity window never goes idle."""
        st_ = {}

        def alloc():
            st_['t'] = psAV.tile([128, 512], F32, tag='av', name='heat')
        items = [(0.0, alloc)]
        for i in range(n):
            def mm():
                nc.tensor.matmul(st_['t'][:], junk[:, 0:128], junk[:],
                                 start=True, stop=True)
            items.append((210.0, mm))
        return items

    # ---- schedule ---------------------------------------------------------
    # V first: ec-major waves keep the PE dense through the DMA trickle.
    # Deterministic heater blocks between the early ec groups fill the
    # chunk-arrival gaps (front-loaded: arrivals accelerate as rings ramp).
    qk = lambda p: qk_items((p,), [(psQ, 'q'), (psQ, 'q')])
    # Merged startup block: wave1 V (st 0-3) + the sc0 halves of Q0/K0,
    # all ec-major — 6 matmuls per arriving (xb, w) ec chunk — plus
    # front-loaded heaters.  qk0-sc0 chains borrow the other 2 psS banks.
    w1 = v_wave_items((0, 1, 2, 3),
                      [(psQ, 'q'), (psQ, 'q'), (psS, 's'), (psS, 's')])
    q0a = qk_items((0,), [(psS, 's'), (psS, 's')], scs=(0,))
    he = heater_items(14)
    he[0][1]()                                   # heater psum alloc
    hmms = [fn for _, fn in he[1:]]
    w1[0][1]()                                   # wave1 psum alloc
    q0a[0][1]()                                  # qk0-sc0 psum alloc
    fills = [4, 3, 3, 2, 1, 1, 0, 0]
    hi = 0
    for i in range(8):
        w1[1 + i][1]()                           # 4 V MMs for this ec
        q0a[1 + i][1]()                          # Q0/K0 sc0 MMs
        for _ in range(fills[i]):
            hmms[hi](); hi += 1
    w1[9][1]()                                   # wave1 copies
    q0a[9][1]()                                  # qk0-sc0 copies
    # wave2 ‖ p0 qc0 (its scores need only the sc0 Q/K halves; AV needs
    # Vall[0..3] from wave1).  wave2 is dense (all data arrived) so it
    # runs 2 chains at a time on psQ, leaving psS to the attention scores.
    a0 = attn_items(0)
    _zip_emit(v_wave_items((4, 5), [(psQ, 'q'), (psQ, 'q')])
              + v_wave_items((6, 7), [(psQ, 'q'), (psQ, 'q')]),
              a0[:9])
    _zip_emit(a0[9:], qk_items((0,), [(psQ, 'q'), (psQ, 'q')], scs=(1,))
              + qk(1))
    _zip_emit(attn_items(1), qk(2))                  # p1 ‖ Q2K2
    _zip_emit(attn_items(2), qk(3), proj_items(0, 128))
    _zip_emit(attn_items(3), proj_items(128, 256))   # p3 ‖ proj(pair 1)
    # trailing half at N=256 (MM 110ns > LDW 97ns: weight loads self-hide);
    # attention psum pools are free now — rotate chains across psQ/psS so
    # copies hide behind the next chains
    for _, fn in proj_items(256, 512, pools=[(psQ, 'q'), (psS, 's')]):
        fn()
    es.close()

    es.close()


_NC_CACHE = None


def _get_nc():
    global _NC_CACHE
    if _NC_CACHE is None:
        _NC_CACHE = build_nc()
    return _NC_CACHE


def make_in_maps(x, Wq, Wk, Wv, W_O):
    import ml_dtypes
    bf = ml_dtypes.bfloat16
    x = np.asarray(x, np.float32)
    xT_by_b = [np.ascontiguousarray(x[b].T.astype(bf)) for b in range(4)]
    W_O = np.ascontiguousarray(np.asarray(W_O, np.float32).T.astype(bf))
    in_maps = []
    for c in range(8):
        b, g = c // 2, c % 2
        hsl = slice(HC * g, HC * g + HC)
        in_maps.append({
            'xb': xT_by_b[b],
            'wq': np.ascontiguousarray(
                np.asarray(Wq, np.float32)[hsl].transpose(1, 0, 2)
                .reshape(E, HC * D).astype(bf)),
            'wk': np.ascontiguousarray(
                np.asarray(Wk, np.float32)[hsl].transpose(1, 0, 2)
                .reshape(E, HC * D).astype(bf)),
            'wv': np.ascontiguousarray(
                np.asarray(Wv, np.float32)[hsl].transpose(1, 0, 2)
                .reshape(E, HC * D).astype(bf)),
            'wo': W_O,
        })
    return in_maps


def kernel(x, Wq, Wk, Wv, W_O):
    from concourse.bass_utils import run_bass_kernel_spmd
    nc = _get_nc()
    in_maps = make_in_maps(x, Wq, Wk, Wv, W_O)
    res = run_bass_kernel_spmd(nc, in_maps, list(range(8)))
    full = np.empty((4, E, E), np.float32)
    for c in range(8):
        b, g = c // 2, c % 2
        full[b, :, NO * g:NO * g + NO] = res.results[c]['out']
    return full
